# revision 1
# baseline (speedup 1.0000x reference)
"""Trainium2 Bass kernel for nn_EventWarping (contrast-maximization event warping loss).

Strategy (data-parallel over batch, one batch element per NeuronCore):
  - flow gather at integer event pixels: indirect DMA (per-event 8B row gather)
  - warped coords wy/wx per event per tref: bulk DVE/ACT math on [128, N/128] tiles
  - the 4 scatter-add histograms (iwe / iwe_ts x pol0 / all) are computed as
    sums of rank-1 outer products of bilinear "hat" row/col vectors:
        hatY[e, y] = relu(1 - |y - wy_e|)   (nonzero at floor(wy), floor(wy)+1)
        hatX[e, x] = relu(1 - |x - wx_e|)
        IWE = sum_e s_e * hatY_e (x) hatX_e  ==  (hatY*s)^T @ hatX  (PSUM accum)
    Out-of-bounds corners drop out automatically (hat is 0 on the [0,H)x[0,W) grid).
  - per-tref loss terms and charbonnier smoothness reduced on device to
    per-partition partial sums; host sums 8 cores' partials.
"""

import os
import sys

import numpy as np

sys.path.insert(0, "/opt/trn_rl_repo")

import concourse.bass as bass
import concourse.bacc as bacc
import concourse.tile as tile
from concourse import mybir
from concourse.alu_op_type import AluOpType as Alu

F32 = mybir.dt.float32
BF16 = mybir.dt.bfloat16
I32 = mybir.dt.int32
AF = mybir.ActivationFunctionType

H, W = 256, 336
SCALE = 336.0
REG_WEIGHT = 0.001
B = 8
N_FULL = 262144
P = 128

# output column layout of the [128, 16] partials tensor
LOSS_COLS = list(range(8))     # 2 trefs x 2 pols x 2 y-halves
SMOOTH_COLS = list(range(8, 14))  # 2 ch x {dxA, dxB, dy}


def build_nc(n_events=N_FULL, group=4, y_on_act=True, mm_dt=BF16, dbg=False, gstage=4):
    """Build the SPMD Bass program for one core / one batch element."""
    cols = n_events // P  # events laid out as [128, cols], chunk = one column
    n_chunks = cols
    assert cols % group == 0

    nc = bacc.Bacc("TRN2", target_bir_lowering=False, debug=False, num_devices=8)

    ev_t = nc.declare_dram_parameter("ev_t", [4, n_events], F32, isOutput=False)
    flow_i = nc.declare_dram_parameter("flow_i", [H * W, 2], F32, isOutput=False)
    flow_r = nc.declare_dram_parameter("flow_r", [2, H, W], F32, isOutput=False)
    out = nc.declare_dram_parameter("out", [P, 16], F32, isOutput=True)
    dbg_t = (
        nc.declare_dram_parameter("dbg", [P, 8 * W], F32, isOutput=True) if dbg else None
    )

    with tile.TileContext(nc) as tc:
        with (
            tc.tile_pool(name="persist", bufs=1) as persist,
            tc.tile_pool(name="tref", bufs=1) as trefp,
            tc.tile_pool(name="grp", bufs=2) as grp,
            tc.tile_pool(name="small", bufs=1) as small,
            tc.tile_pool(name="psum", bufs=1, space="PSUM") as psum,
        ):
            # ---------------- constants ----------------
            iota_i = small.tile([P, W], I32, tag="iota_i")
            nc.gpsimd.iota(iota_i[:], pattern=[[1, W]], base=0, channel_multiplier=0)
            # value (1 - k) and (k + 1) tiles for hat construction
            yc1 = small.tile([P, H], F32, tag="yc1")
            yc2 = small.tile([P, H], F32, tag="yc2")
            xc1 = small.tile([P, W], F32, tag="xc1")
            xc2 = small.tile([P, W], F32, tag="xc2")
            nc.vector.tensor_scalar(yc1[:], iota_i[:, :H], -1.0, 1.0, Alu.mult, Alu.add)
            nc.vector.tensor_scalar(yc2[:], iota_i[:, :H], 1.0, None, Alu.add)
            nc.vector.tensor_scalar(xc1[:], iota_i[:], -1.0, 1.0, Alu.mult, Alu.add)
            nc.vector.tensor_scalar(xc2[:], iota_i[:], 1.0, None, Alu.add)

            # ---------------- output partials tile ----------------
            out_t = small.tile([P, 16], F32, tag="out_t")
            nc.vector.memset(out_t[:], 0.0)

            c1em6 = small.tile([P, 1], F32, tag="c1em6")
            nc.vector.memset(c1em6[:], 1e-6)

            # ---------------- load event scalars ----------------
            def load_row(r, tag):
            # ev_t row r [n_events] -> [128, cols]
                t = persist.tile([P, cols], F32, tag=tag)
                nc.sync.dma_start(out=t[:], in_=ev_t[r].rearrange("(p c) -> p c", p=P))
                return t

            ts_t = load_row(0, "ts")
            ys_t = load_row(1, "ys")
            xs_t = load_row(2, "xs")
            pm0_t = trefp.tile([P, cols], F32, tag="m1", name="pm0_t")
            nc.sync.dma_start(out=pm0_t[:], in_=ev_t[3].rearrange("(p c) -> p c", p=P))

            pm0b = persist.tile([P, cols], BF16, tag="pm0b")
            nc.vector.tensor_copy(pm0b[:], pm0_t[:])

            # ---------------- flow gather ----------------
            # indirect_dma_start semantics on HW: one index per output
            # partition-row; out[p, :] = in.flat[idx[p, 0]*coef : ...].
            # So gather one event-column (128 events) per instruction.
            pixi = persist.tile([P, cols], I32, tag="pixi")
            nc.vector.scalar_tensor_tensor(
                pixi[:], ys_t[:], float(W), xs_t[:], Alu.mult, Alu.add
            )

            fg = persist.tile([P, 2 * cols], F32, tag="fg")
            if gstage < 2:
                nc.vector.memset(fg[:], 0.0001)
            for c in range(cols if gstage >= 2 else 0):
                nc.gpsimd.indirect_dma_start(
                    out=fg[:, 2 * c : 2 * c + 2],
                    out_offset=None,
                    in_=flow_i[:],
                    in_offset=bass.IndirectOffsetOnAxis(ap=pixi[:, c : c + 1], axis=0),
                )

            fy_t = fg[:].rearrange("p (c two) -> p c two", two=2)[:, :, 0]
            fx_t = fg[:].rearrange("p (c two) -> p c two", two=2)[:, :, 1]

            # ---------------- charbonnier smoothness ----------------
            # per channel: F [128, 672] rows (2p, 2p+1); Fs [127, 672] rows (2p+1, 2p+2)
            smooth_scratch = small.tile([P, 672], F32, tag="smooth_scratch")
            for ch in range(2):
                Fc = small.tile([P, 2 * W], F32, tag="Fc")
                nc.sync.dma_start(out=Fc[:], in_=flow_r[ch].rearrange("(p r) w -> p (r w)", r=2))
                Fs = small.tile([P - 1, 2 * W], F32, tag="Fs")
                nc.sync.dma_start(
                    out=Fs[:],
                    in_=flow_r[ch, 1 : 2 * P - 1, :].rearrange("(p r) w -> p (r w)", r=2),
                )
                # dxA: rows 2p - (2p+1); all 128 partitions
                dxa = small.tile([P, W], F32, tag="dxa")
                nc.vector.tensor_tensor(dxa[:], Fc[:, 0:W], Fc[:, W : 2 * W], Alu.subtract)
                nc.vector.scalar_tensor_tensor(
                    smooth_scratch[:, 0:W], dxa[:], 0.0, dxa[:], Alu.add, Alu.mult
                )
                nc.scalar.activation(
                    smooth_scratch[:, 0:W], smooth_scratch[:, 0:W], AF.Sqrt,
                    bias=c1em6[:], scale=1.0,
                    accum_out=out_t[:, 8 + 3 * ch : 9 + 3 * ch],
                )
                # dxB: rows (2p+1) - (2p+2); 127 partitions
                dxb = small.tile([P - 1, W], F32, tag="dxb")
                nc.vector.tensor_tensor(
                    dxb[:], Fc[: P - 1, W : 2 * W], Fs[:, W : 2 * W], Alu.subtract
                )
                nc.vector.scalar_tensor_tensor(
                    smooth_scratch[: P - 1, 0:W], dxb[:], 0.0, dxb[:], Alu.add, Alu.mult
                )
                nc.scalar.activation(
                    smooth_scratch[: P - 1, 0:W], smooth_scratch[: P - 1, 0:W], AF.Sqrt,
                    bias=c1em6[:P-1], scale=1.0,
                    accum_out=out_t[: P - 1, 9 + 3 * ch : 10 + 3 * ch],
                )
                # dy: within-row x-diffs, 2 blocks of 335 per partition
                dy = small.tile([P, 2 * (W - 1)], F32, tag="dy")
                src_a = Fc[:].rearrange("p (r w) -> p r w", r=2)[:, :, 0 : W - 1]
                src_b = Fc[:].rearrange("p (r w) -> p r w", r=2)[:, :, 1:W]
                nc.vector.tensor_tensor(dy[:].rearrange("p (r w) -> p r w", r=2), src_a, src_b, Alu.subtract)
                nc.vector.scalar_tensor_tensor(
                    smooth_scratch[:, 0 : 2 * (W - 1)], dy[:], 0.0, dy[:], Alu.add, Alu.mult
                )
                nc.scalar.activation(
                    smooth_scratch[:, 0 : 2 * (W - 1)], smooth_scratch[:, 0 : 2 * (W - 1)],
                    AF.Sqrt, bias=c1em6[:], scale=1.0,
                    accum_out=out_t[:, 10 + 3 * ch : 11 + 3 * ch],
                )

            # ---------------- per-tref pipeline ----------------
            for it, tref in enumerate((1.0, 0.0)):
                # bulk per-event math
                wy = trefp.tile([P, cols], F32, tag="wy")
                wx = trefp.tile([P, cols], F32, tag="wx")
                m1 = trefp.tile([P, cols], F32, tag="m1")
                # m1 = (ts - tref) * f; w = m1 * (-SCALE) + base
                nc.vector.scalar_tensor_tensor(m1[:], ts_t[:], float(tref), fy_t, Alu.subtract, Alu.mult)
                nc.vector.scalar_tensor_tensor(wy[:], m1[:], -SCALE, ys_t[:], Alu.mult, Alu.add)
                nc.vector.scalar_tensor_tensor(m1[:], ts_t[:], float(tref), fx_t, Alu.subtract, Alu.mult)
                nc.vector.scalar_tensor_tensor(wx[:], m1[:], -SCALE, xs_t[:], Alu.mult, Alu.add)

                tgb = trefp.tile([P, cols], BF16, tag="tgb")
                if tref == 1.0:
                    nc.vector.tensor_copy(tgb[:], ts_t[:])
                else:
                    nc.vector.tensor_scalar(tgb[:], ts_t[:], -1.0, 1.0, Alu.mult, Alu.add)

                if y_on_act:
                    wyn = trefp.tile([P, cols], F32, tag="wyn")
                    nc.vector.tensor_scalar(wyn[:], wy[:], -1.0, None, Alu.mult)

                # 8 persistent psum accumulators: (ALL, P0, TSALL, TSP0) x y-half
                ps = {}
                for v in ("all", "p0", "tsall", "tsp0"):
                    for h in range(2):
                        ps[(v, h)] = psum.tile([P, W], F32, tag=f"ps_{v}_{h}", name=f"ps_{v}_{h}")

                n_groups = n_chunks // group
                for g in range(n_groups):
                    c0 = g * group
                    raY = grp.tile([P, group * H], BF16, tag="raY")
                    rbY = grp.tile([P, group * H], BF16, tag="rbY")
                    raX = grp.tile([P, group * W], BF16, tag="raX")
                    rbX = grp.tile([P, group * W], BF16, tag="rbX")
                    for j in range(group):
                        c = c0 + j
                        sy = slice(j * H, (j + 1) * H)
                        sx = slice(j * W, (j + 1) * W)
                        if y_on_act:
                            nc.scalar.activation(
                                raY[:, sy], yc1[:], AF.Relu, bias=wy[:, c : c + 1], scale=1.0
                            )
                            nc.scalar.activation(
                                rbY[:, sy], yc2[:], AF.Relu, bias=wyn[:, c : c + 1], scale=1.0
                            )
                        else:
                            nc.vector.tensor_scalar(
                                raY[:, sy], yc1[:], wy[:, c : c + 1], 0.0, Alu.add, Alu.max
                            )
                            nc.vector.tensor_scalar(
                                rbY[:, sy], yc2[:], wy[:, c : c + 1], 0.0, Alu.subtract, Alu.max
                            )
                        nc.vector.tensor_scalar(
                            raX[:, sx], xc1[:], wx[:, c : c + 1], 0.0, Alu.add, Alu.max
                        )
                        nc.vector.tensor_scalar(
                            rbX[:, sx], xc2[:], wx[:, c : c + 1], 0.0, Alu.subtract, Alu.max
                        )
                    hatY = grp.tile([P, group * H], BF16, tag="hatY")
                    hatX = grp.tile([P, group * W], BF16, tag="hatX")
                    hatYtg = grp.tile([P, group * H], BF16, tag="hatYtg")
                    hatXpm = grp.tile([P, group * W], BF16, tag="hatXpm")
                    nc.vector.tensor_tensor(hatY[:], raY[:], rbY[:], Alu.min)
                    nc.vector.tensor_tensor(hatX[:], raX[:], rbX[:], Alu.min)
                    nc.vector.tensor_tensor(
                        hatYtg[:].rearrange("p (g y) -> p g y", g=group),
                        hatY[:].rearrange("p (g y) -> p g y", g=group),
                        tgb[:, c0 : c0 + group].to_broadcast([P, group, H]),
                        Alu.mult,
                    )
                    nc.vector.tensor_tensor(
                        hatXpm[:].rearrange("p (g x) -> p g x", g=group),
                        hatX[:].rearrange("p (g x) -> p g x", g=group),
                        pm0b[:, c0 : c0 + group].to_broadcast([P, group, W]),
                        Alu.mult,
                    )
                    for j in range(group):
                        c = c0 + j
                        start = c == 0
                        stop = c == n_chunks - 1
                        sx = slice(j * W, (j + 1) * W)
                        for h in range(2):
                            sh = slice(j * H + h * P, j * H + (h + 1) * P)
                            for v, lhs, rhs in (
                                ("all", hatY, hatX),
                                ("p0", hatY, hatXpm),
                                ("tsall", hatYtg, hatX),
                                ("tsp0", hatYtg, hatXpm),
                            ):
                                nc.tensor.matmul(
                                    ps[(v, h)][:],
                                    lhsT=lhs[:, sh],
                                    rhs=rhs[:, sx],
                                    start=start,
                                    stop=stop,
                                )

                # loss terms from accumulated images
                for h in range(2):
                    s_all = small.tile([P, W], F32, tag="s_all")
                    s_p0 = small.tile([P, W], F32, tag="s_p0")
                    t_all = small.tile([P, W], F32, tag="t_all")
                    t_p0 = small.tile([P, W], F32, tag="t_p0")
                    nc.vector.tensor_copy(s_all[:], ps[("all", h)][:])
                    nc.vector.tensor_copy(s_p0[:], ps[("p0", h)][:])
                    nc.vector.tensor_copy(t_all[:], ps[("tsall", h)][:])
                    nc.vector.tensor_copy(t_p0[:], ps[("tsp0", h)][:])
                    if dbg and it == 0:
                        for di, timg in enumerate((s_all, s_p0, t_all, t_p0)):
                            nc.sync.dma_start(
                                out=dbg_t[:, (4 * h + di) * W : (4 * h + di + 1) * W],
                                in_=timg[:],
                            )
                    # pol1 = all - pol0 (in place into s_all/t_all)
                    nc.vector.tensor_tensor(s_all[:], s_all[:], s_p0[:], Alu.subtract)
                    nc.vector.tensor_tensor(t_all[:], t_all[:], t_p0[:], Alu.subtract)
                    for pi, (S_img, T_img) in enumerate(((s_p0, t_p0), (s_all, t_all))):
                        r = small.tile([P, W], F32, tag="recip")
                        nc.vector.tensor_scalar(r[:], S_img[:], 1e-9, None, Alu.add)
                        nc.vector.reciprocal(r[:], r[:])
                        q = small.tile([P, W], F32, tag="q")
                        nc.vector.tensor_tensor(q[:], T_img[:], r[:], Alu.mult)
                        col = 4 * it + 2 * h + pi
                        nc.vector.scalar_tensor_tensor(
                            r[:], q[:], 0.0, q[:], Alu.add, Alu.mult,
                            accum_out=out_t[:, col : col + 1],
                        )

            nc.sync.dma_start(out=out[:], in_=out_t[:])

    nc.finalize()
    return nc


# ---------------------------------------------------------------------------
# host-side wrapper
# ---------------------------------------------------------------------------

_CACHED = {}


def _get_nc():
    key = "full"
    if key not in _CACHED:
        _CACHED[key] = build_nc()
    return _CACHED[key]


def prep_core_inputs(flow_b, ev_b, pm_b):
    """Per-batch-element host prep: pure re-layout (sharding), no math beyond layout."""
    n = ev_b.shape[0]
    ev_t = np.empty((4, n), dtype=np.float32)
    ev_t[0] = ev_b[:, 0]
    ev_t[1] = ev_b[:, 1]
    ev_t[2] = ev_b[:, 2]
    ev_t[3] = pm_b[:, 0]
    flow_i = np.ascontiguousarray(
        np.stack([flow_b[1].reshape(-1), flow_b[0].reshape(-1)], axis=-1)
    ).astype(np.float32)
    flow_r = np.ascontiguousarray(flow_b).astype(np.float32)
    return {"ev_t": ev_t, "flow_i": flow_i, "flow_r": flow_r}


def finish(outs):
    """Combine per-core partials into the scalar loss."""
    total = np.float64(0.0)
    for o in outs:
        o = o.astype(np.float64)
        total += o[:, LOSS_COLS].sum() + REG_WEIGHT * o[:, SMOOTH_COLS].sum()
    return np.float32(total)


def kernel(flow, event_list, pol_mask):
    from concourse.bass_utils import run_bass_kernel_spmd

    flow = np.asarray(flow)
    event_list = np.asarray(event_list)
    pol_mask = np.asarray(pol_mask)
    nc = _get_nc()
    in_maps = [
        prep_core_inputs(flow[b], event_list[b], pol_mask[b]) for b in range(B)
    ]
    res = run_bass_kernel_spmd(nc, in_maps, list(range(B)))
    outs = [res.results[b]["out"] for b in range(B)]
    return finish(outs)


if __name__ == "__main__":
    # smoke test with random data
    rng = np.random.default_rng(0)
    flow = (0.05 * rng.standard_normal((B, 2, H, W))).astype(np.float32)
    ys = rng.integers(0, H, (B, N_FULL)).astype(np.float32)
    xs = rng.integers(0, W, (B, N_FULL)).astype(np.float32)
    ts = rng.random((B, N_FULL), dtype=np.float32)
    pol = rng.integers(0, 2, (B, N_FULL))
    ev = np.stack([ts, ys, xs, pol * 2.0 - 1.0], axis=-1).astype(np.float32)
    pm = np.stack([(pol == 1), (pol == 0)], axis=-1).astype(np.float32)
    print(kernel(flow, ev, pm))



# revision 3
# speedup vs baseline: 13.9733x; 13.9733x over previous
"""Trainium2 Bass kernel for nn_EventWarping (contrast-maximization event warping loss).

Strategy (data-parallel over batch, one batch element per NeuronCore):
  - flow gather at integer event pixels: per-column indirect DMA (128 events
    each), emitted per block so compute pipelines behind the gather stream.
  - hat construction in fp16 via 2-stage form:
        u  = |iota - w|          (ACT Abs, per-partition bias = -w)
        nh = min(u, 1) - 1       (DVE 2-op tensor_scalar, = -hat, in [-1, 0])
    Negations cancel in the outer products (all four factor tiles are <= 0),
    so all accumulated images come out positive.
  - per-event scalar weights (tg = timestamp grad, pm = polarity mask) fold in
    via fp16 scalar-ptr multiplies (f32 [128,1] scalar operand keeps DVE 2x).
  - 4 histograms per tref (all / p0 / ts*all / ts*p0) accumulate as rank-1
    outer products into 8 persistent PSUM banks: IWE = (nY)^T @ nX etc.
  - loss terms + charbonnier smoothness reduced on device to per-partition
    partial sums; host sums 8 cores' partials.
"""

import os
import sys

import numpy as np

sys.path.insert(0, "/opt/trn_rl_repo")

import concourse.bass as bass
import concourse.bacc as bacc
import concourse.tile as tile
from concourse import mybir
from concourse.alu_op_type import AluOpType as Alu

F32 = mybir.dt.float32
F16 = mybir.dt.float16
I32 = mybir.dt.int32
AF = mybir.ActivationFunctionType

H, W = 256, 336
SCALE = 336.0
REG_WEIGHT = 0.001
B = 8
N_FULL = 262144
P = 128

# output column layout of the [128, 16] partials tensor
LOSS_COLS = list(range(8))     # 2 trefs x 2 pols x 2 y-halves
SMOOTH_COLS = list(range(8, 14))  # 2 ch x {dxA, dxB, dy}


def build_nc(n_events=N_FULL, group=8, blk=128):
    """Build the SPMD Bass program for one core / one batch element."""
    cols = n_events // P  # events laid out as [128, cols], chunk = one column
    assert cols % blk == 0 and blk % group == 0
    n_blocks = cols // blk

    nc = bacc.Bacc("TRN2", target_bir_lowering=False, debug=False, num_devices=8)

    ev_t = nc.declare_dram_parameter("ev_t", [4, n_events], F32, isOutput=False)
    flow_i = nc.declare_dram_parameter("flow_i", [H * W, 2], F32, isOutput=False)
    flow_r = nc.declare_dram_parameter("flow_r", [2, H, W], F32, isOutput=False)
    out = nc.declare_dram_parameter("out", [P, 16], F32, isOutput=True)

    with tile.TileContext(nc) as tc:
        with (
            tc.tile_pool(name="persist", bufs=1) as persist,
            tc.tile_pool(name="blkp", bufs=3) as blkp,
            tc.tile_pool(name="grp", bufs=3) as grp,
            tc.tile_pool(name="small", bufs=1) as small,
            tc.tile_pool(name="psum", bufs=1, space="PSUM") as psum,
        ):
            # ---------------- constants ----------------
            iota_i = small.tile([P, W], I32, tag="iota_i")
            nc.gpsimd.iota(iota_i[:], pattern=[[1, W]], base=0, channel_multiplier=0)
            yio16 = small.tile([P, H], F16, tag="yio16")
            xio16 = small.tile([P, W], F16, tag="xio16")
            nc.vector.tensor_copy(yio16[:], iota_i[:, :H])
            nc.vector.tensor_copy(xio16[:], iota_i[:])

            out_t = small.tile([P, 16], F32, tag="out_t")
            nc.vector.memset(out_t[:], 0.0)
            c1em6 = small.tile([P, 1], F32, tag="c1em6")
            nc.vector.memset(c1em6[:], 1e-6)

            # ---------------- load event scalars ----------------
            def load_row(r, tag):
                t = persist.tile([P, cols], F32, tag=tag)
                nc.sync.dma_start(out=t[:], in_=ev_t[r].rearrange("(p c) -> p c", p=P))
                return t

            ts_t = load_row(0, "ts")
            ys_t = load_row(1, "ys")
            xs_t = load_row(2, "xs")
            pm_t = load_row(3, "pm")

            # tg for tref=0 is (1 - ts); tref=1 uses ts directly
            tg0 = persist.tile([P, cols], F32, tag="tg0")
            nc.vector.tensor_scalar(tg0[:], ts_t[:], -1.0, 1.0, Alu.mult, Alu.add)

            # ---------------- flow gather (per block, pipelined) ----------------
            pixi = persist.tile([P, cols], I32, tag="pixi")
            nc.vector.scalar_tensor_tensor(
                pixi[:], ys_t[:], float(W), xs_t[:], Alu.mult, Alu.add
            )
            fgb = []
            for b in range(n_blocks):
                fg = persist.tile([P, 2 * blk], F32, tag=f"fg{b}", name=f"fg{b}")
                fgb.append(fg)
                for j in range(blk):
                    c = b * blk + j
                    nc.gpsimd.indirect_dma_start(
                        out=fg[:, 2 * j : 2 * j + 2],
                        out_offset=None,
                        in_=flow_i[:],
                        in_offset=bass.IndirectOffsetOnAxis(
                            ap=pixi[:, c : c + 1], axis=0
                        ),
                    )

            # ---------------- charbonnier smoothness ----------------
            smooth_scratch = small.tile([P, 672], F32, tag="smooth_scratch")
            for ch in range(2):
                Fc = small.tile([P, 2 * W], F32, tag="Fc")
                nc.sync.dma_start(
                    out=Fc[:], in_=flow_r[ch].rearrange("(p r) w -> p (r w)", r=2)
                )
                Fs = small.tile([P - 1, 2 * W], F32, tag="Fs")
                nc.sync.dma_start(
                    out=Fs[:],
                    in_=flow_r[ch, 1 : 2 * P - 1, :].rearrange("(p r) w -> p (r w)", r=2),
                )
                dxa = small.tile([P, W], F32, tag="dxa")
                nc.vector.tensor_tensor(dxa[:], Fc[:, 0:W], Fc[:, W : 2 * W], Alu.subtract)
                nc.vector.scalar_tensor_tensor(
                    smooth_scratch[:, 0:W], dxa[:], 0.0, dxa[:], Alu.add, Alu.mult
                )
                nc.scalar.activation(
                    smooth_scratch[:, 0:W], smooth_scratch[:, 0:W], AF.Sqrt,
                    bias=c1em6[:], scale=1.0,
                    accum_out=out_t[:, 8 + 3 * ch : 9 + 3 * ch],
                )
                dxb = small.tile([P - 1, W], F32, tag="dxb")
                nc.vector.tensor_tensor(
                    dxb[:], Fc[: P - 1, W : 2 * W], Fs[:, W : 2 * W], Alu.subtract
                )
                nc.vector.scalar_tensor_tensor(
                    smooth_scratch[: P - 1, 0:W], dxb[:], 0.0, dxb[:], Alu.add, Alu.mult
                )
                nc.scalar.activation(
                    smooth_scratch[: P - 1, 0:W], smooth_scratch[: P - 1, 0:W], AF.Sqrt,
                    bias=c1em6[: P - 1], scale=1.0,
                    accum_out=out_t[: P - 1, 9 + 3 * ch : 10 + 3 * ch],
                )
                dy = small.tile([P, 2 * (W - 1)], F32, tag="dy")
                src_a = Fc[:].rearrange("p (r w) -> p r w", r=2)[:, :, 0 : W - 1]
                src_b = Fc[:].rearrange("p (r w) -> p r w", r=2)[:, :, 1:W]
                nc.vector.tensor_tensor(
                    dy[:].rearrange("p (r w) -> p r w", r=2), src_a, src_b, Alu.subtract
                )
                nc.vector.scalar_tensor_tensor(
                    smooth_scratch[:, 0 : 2 * (W - 1)], dy[:], 0.0, dy[:], Alu.add, Alu.mult
                )
                nc.scalar.activation(
                    smooth_scratch[:, 0 : 2 * (W - 1)], smooth_scratch[:, 0 : 2 * (W - 1)],
                    AF.Sqrt, bias=c1em6[:], scale=1.0,
                    accum_out=out_t[:, 10 + 3 * ch : 11 + 3 * ch],
                )

            # ---------------- per-tref pipeline ----------------
            for it, tref in enumerate((1.0, 0.0)):
                tg = ts_t if tref == 1.0 else tg0

                # 8 persistent psum accumulators: (ALL, P0, TSALL, TSP0) x y-half
                ps = {}
                for v in ("all", "p0", "tsall", "tsp0"):
                    for h in range(2):
                        ps[(v, h)] = psum.tile(
                            [P, W], F32, tag=f"ps_{v}_{h}", name=f"ps_{v}_{h}"
                        )

                for b in range(n_blocks):
                    c0b = b * blk
                    fy = fgb[b][:].rearrange("p (c two) -> p c two", two=2)[:, :, 0]
                    fx = fgb[b][:].rearrange("p (c two) -> p c two", two=2)[:, :, 1]
                    sb = slice(c0b, c0b + blk)
                    # negw = m1*SCALE - base  (= -w), one STT each after m1
                    m1 = blkp.tile([P, blk], F32, tag="m1")
                    negwy = blkp.tile([P, blk], F32, tag="negwy")
                    negwx = blkp.tile([P, blk], F32, tag="negwx")
                    nc.vector.scalar_tensor_tensor(
                        m1[:], ts_t[:, sb], float(tref), fy, Alu.subtract, Alu.mult
                    )
                    nc.vector.scalar_tensor_tensor(
                        negwy[:], m1[:], SCALE, ys_t[:, sb], Alu.mult, Alu.subtract
                    )
                    nc.vector.scalar_tensor_tensor(
                        m1[:], ts_t[:, sb], float(tref), fx, Alu.subtract, Alu.mult
                    )
                    nc.vector.scalar_tensor_tensor(
                        negwx[:], m1[:], SCALE, xs_t[:, sb], Alu.mult, Alu.subtract
                    )

                    for g in range(blk // group):
                        c0 = c0b + g * group
                        j0 = g * group
                        uY = grp.tile([P, group * H], F16, tag="uY")
                        uX = grp.tile([P, group * W], F16, tag="uX")
                        for j in range(group):
                            jj = j0 + j
                            nc.scalar.activation(
                                uY[:, j * H : (j + 1) * H], yio16[:], AF.Abs,
                                bias=negwy[:, jj : jj + 1], scale=1.0,
                            )
                            nc.scalar.activation(
                                uX[:, j * W : (j + 1) * W], xio16[:], AF.Abs,
                                bias=negwx[:, jj : jj + 1], scale=1.0,
                            )
                        # negated hats: nh = min(u,1) - 1  in [-1, 0]
                        nY = grp.tile([P, group * H], F16, tag="nY")
                        nX = grp.tile([P, group * W], F16, tag="nX")
                        nc.vector.tensor_scalar(
                            nY[:], uY[:], 1.0, 1.0, Alu.min, Alu.subtract
                        )
                        nc.vector.tensor_scalar(
                            nX[:], uX[:], 1.0, 1.0, Alu.min, Alu.subtract
                        )
                        nYtg = grp.tile([P, group * H], F16, tag="nYtg")
                        nXpm = grp.tile([P, group * W], F16, tag="nXpm")
                        for j in range(group):
                            c = c0 + j
                            sy = slice(j * H, (j + 1) * H)
                            sx = slice(j * W, (j + 1) * W)
                            nc.vector.tensor_scalar(
                                nYtg[:, sy], nY[:, sy], tg[:, c : c + 1], None, Alu.mult
                            )
                            nc.vector.tensor_scalar(
                                nXpm[:, sx], nX[:, sx], pm_t[:, c : c + 1], None, Alu.mult
                            )
                        for j in range(group):
                            c = c0 + j
                            start = c == 0
                            stop = c == cols - 1
                            sx = slice(j * W, (j + 1) * W)
                            for h in range(2):
                                sh = slice(j * H + h * P, j * H + (h + 1) * P)
                                for v, lhs, rhs in (
                                    ("all", nY, nX),
                                    ("p0", nY, nXpm),
                                    ("tsall", nYtg, nX),
                                    ("tsp0", nYtg, nXpm),
                                ):
                                    nc.tensor.matmul(
                                        ps[(v, h)][:],
                                        lhsT=lhs[:, sh],
                                        rhs=rhs[:, sx],
                                        start=start,
                                        stop=stop,
                                    )

                # loss terms from accumulated images (all positive)
                for h in range(2):
                    s_all = small.tile([P, W], F32, tag="s_all")
                    t_all = small.tile([P, W], F32, tag="t_all")
                    nc.vector.tensor_copy(s_all[:], ps[("all", h)][:])
                    nc.vector.tensor_copy(t_all[:], ps[("tsall", h)][:])
                    s1 = small.tile([P, W], F32, tag="s1")
                    t1 = small.tile([P, W], F32, tag="t1")
                    nc.vector.tensor_tensor(
                        s1[:], s_all[:], ps[("p0", h)][:], Alu.subtract
                    )
                    nc.vector.tensor_tensor(
                        t1[:], t_all[:], ps[("tsp0", h)][:], Alu.subtract
                    )
                    for pi, (S_img, T_img) in enumerate(
                        ((ps[("p0", h)], ps[("tsp0", h)]), (s1, t1))
                    ):
                        r = small.tile([P, W], F32, tag="recip")
                        nc.vector.tensor_scalar(r[:], S_img[:], 1e-9, None, Alu.add)
                        nc.vector.reciprocal(r[:], r[:])
                        q = small.tile([P, W], F32, tag="q")
                        nc.vector.tensor_tensor(q[:], T_img[:], r[:], Alu.mult)
                        col = 4 * it + 2 * h + pi
                        nc.vector.scalar_tensor_tensor(
                            r[:], q[:], 0.0, q[:], Alu.add, Alu.mult,
                            accum_out=out_t[:, col : col + 1],
                        )

            nc.sync.dma_start(out=out[:], in_=out_t[:])

    nc.finalize()
    return nc


# ---------------------------------------------------------------------------
# host-side wrapper
# ---------------------------------------------------------------------------

_CACHED = {}


def _get_nc():
    key = "full"
    if key not in _CACHED:
        _CACHED[key] = build_nc()
    return _CACHED[key]


def prep_core_inputs(flow_b, ev_b, pm_b):
    """Per-batch-element host prep: pure re-layout (sharding), no math beyond layout."""
    n = ev_b.shape[0]
    ev_t = np.empty((4, n), dtype=np.float32)
    ev_t[0] = ev_b[:, 0]
    ev_t[1] = ev_b[:, 1]
    ev_t[2] = ev_b[:, 2]
    ev_t[3] = pm_b[:, 0]
    flow_i = np.ascontiguousarray(
        np.stack([flow_b[1].reshape(-1), flow_b[0].reshape(-1)], axis=-1)
    ).astype(np.float32)
    flow_r = np.ascontiguousarray(flow_b).astype(np.float32)
    return {"ev_t": ev_t, "flow_i": flow_i, "flow_r": flow_r}


def finish(outs):
    """Combine per-core partials into the scalar loss."""
    total = np.float64(0.0)
    for o in outs:
        o = o.astype(np.float64)
        total += o[:, LOSS_COLS].sum() + REG_WEIGHT * o[:, SMOOTH_COLS].sum()
    return np.float32(total)


def kernel(flow, event_list, pol_mask):
    from concourse.bass_utils import run_bass_kernel_spmd

    flow = np.asarray(flow)
    event_list = np.asarray(event_list)
    pol_mask = np.asarray(pol_mask)
    nc = _get_nc()
    in_maps = [
        prep_core_inputs(flow[b], event_list[b], pol_mask[b]) for b in range(B)
    ]
    res = run_bass_kernel_spmd(nc, in_maps, list(range(B)))
    outs = [res.results[b]["out"] for b in range(B)]
    return finish(outs)


if __name__ == "__main__":
    rng = np.random.default_rng(0)
    flow = (0.05 * rng.standard_normal((B, 2, H, W))).astype(np.float32)
    ys = rng.integers(0, H, (B, N_FULL)).astype(np.float32)
    xs = rng.integers(0, W, (B, N_FULL)).astype(np.float32)
    ts = rng.random((B, N_FULL), dtype=np.float32)
    pol = rng.integers(0, 2, (B, N_FULL))
    ev = np.stack([ts, ys, xs, pol * 2.0 - 1.0], axis=-1).astype(np.float32)
    pm = np.stack([(pol == 1), (pol == 0)], axis=-1).astype(np.float32)
    print(kernel(flow, ev, pm))
